# revision 6
# baseline (speedup 1.0000x reference)
"""Trainium2 Bass kernel for DiagTrainableLDAHead (retrieval_knn).

out[n,c] = log_prior[c] - 0.5*(m2[n,c] + log_det)
m2[n,c]  = sum_d (z[n,d]-mu[c,d])^2 * inv_var[d]
         = z_sq[n] - 2*cross[n,c] + mu_sq[c]

=> out[n,c] = cross[n,c] + rb[n] + cb[c]
   cross = z @ (mu * inv_var).T            (GEMM, fp32r single-pass)
   rb[n] = -0.5 * sum_d z[n,d]^2 inv_var[d]
   cb[c] = log_prior[c] - 0.5*(mu_sq[c] + log_det)

Sharding: data-parallel over N across 8 NeuronCores (1024 rows each);
mu / log_cov_diag / prior_logits replicated. Forward-only: no collectives.
Host prep is layout-only (transposes so the contraction dim D sits on
SBUF partitions for both GEMM operands); all arithmetic is on-device.
"""
import sys

sys.path.insert(0, "/opt/trn_rl_repo")

import numpy as np

import concourse.bacc as bacc
import concourse.tile as tile
from concourse import mybir
from concourse.bass_utils import run_bass_kernel_spmd

F32 = mybir.dt.float32
F32R = mybir.dt.float32r
AF = mybir.ActivationFunctionType
ALU = mybir.AluOpType

N, C, D = 8192, 2048, 512
NCORES = 8
NSH = N // NCORES          # 1024 rows per core
P = 128                    # partitions
KT = D // P                # 4 k-tiles
NT = NSH // P              # 8 n-tiles
F = 512                    # c-chunk (PSUM bank / fp32 moving max)
CJ = C // F                # 4 c-chunks

_CACHE = {}


def _build():
    nc = bacc.Bacc("TRN2", target_bir_lowering=False, debug=False,
                   enable_asserts=False, num_devices=NCORES)

    zT = nc.dram_tensor("zT", [D, NSH], F32, kind="ExternalInput").ap()
    muT = nc.dram_tensor("muT", [D, C], F32, kind="ExternalInput").ap()
    lc = nc.dram_tensor("lc", [D], F32, kind="ExternalInput").ap()
    prior = nc.dram_tensor("prior", [C], F32, kind="ExternalInput").ap()
    out = nc.dram_tensor("out", [NSH, C], F32, kind="ExternalOutput").ap()

    with tile.TileContext(nc) as tc:
        with (
            tc.tile_pool(name="const", bufs=1) as const,
            tc.tile_pool(name="rot", bufs=2) as rot,
            tc.tile_pool(name="sq", bufs=2) as sq,
            tc.tile_pool(name="stage", bufs=3) as stage,
            tc.tile_pool(name="psS", bufs=2, space="PSUM") as psS,
            tc.tile_pool(name="psM", bufs=4, space="PSUM") as psM,
        ):
            # ---- small constants --------------------------------------
            # log_cov in partition layout [p, t] with d = t*128 + p
            lc_p = const.tile([P, KT], F32)
            nc.sync.dma_start(out=lc_p[:], in_=lc.rearrange("(t p) -> p t", p=P))
            iv = const.tile([P, KT], F32)      # exp(-lc), for scalar ops
            nc.scalar.activation(iv[:], lc_p[:], AF.Exp, scale=-1.0)
            iv_r = const.tile([P, KT], F32R)   # rounded copy, matmul operand
            nc.scalar.activation(iv_r[:], lc_p[:], AF.Exp, scale=-1.0)
            # [iv, 0] pairs: fp32r matmul needs moving free dim >= 2
            iv2 = const.tile([P, KT, 2], F32R)
            nc.scalar.activation(iv2[:, :, 0:1], lc_p[:, :, None], AF.Exp,
                                 scale=-1.0)
            nc.scalar.mul(iv2[:, :, 1:2], lc_p[:, :, None], 0.0)

            # log_det = sum(lc); computed exactly along the free dim
            lc_f = const.tile([1, D], F32)
            nc.sync.dma_start(out=lc_f[:], in_=lc.rearrange("(a d) -> a d", a=1))
            ldsum = const.tile([1, 1], F32)
            nc.vector.tensor_reduce(out=ldsum[:], in_=lc_f[:],
                                    axis=mybir.AxisListType.X, op=ALU.add)
            nldh = const.tile([1, 1], F32)     # -0.5 * log_det
            nc.scalar.mul(nldh[:], ldsum[:], -0.5)

            # log_prior = prior - max - log(sum(exp(prior - max)))
            pr = const.tile([1, C], F32)
            nc.sync.dma_start(out=pr[:], in_=prior.rearrange("(a c) -> a c", a=1))
            pmax = const.tile([1, 1], F32)
            nc.vector.tensor_reduce(out=pmax[:], in_=pr[:],
                                    axis=mybir.AxisListType.X, op=ALU.max)
            npmax = const.tile([1, 1], F32)
            nc.scalar.mul(npmax[:], pmax[:], -1.0)
            pexp = const.tile([1, C], F32)
            nc.scalar.activation(pexp[:], pr[:], AF.Exp, bias=npmax[:], scale=1.0)
            psum_e = const.tile([1, 1], F32)
            nc.vector.tensor_reduce(out=psum_e[:], in_=pexp[:],
                                    axis=mybir.AxisListType.X, op=ALU.add)
            lse = const.tile([1, 1], F32)
            nc.scalar.activation(lse[:], psum_e[:], AF.Ln)
            nb = const.tile([1, 1], F32)       # -(lse + pmax)
            nc.scalar.activation(nb[:], lse[:], AF.Identity, bias=pmax[:], scale=1.0)
            nc.scalar.mul(nb[:], nb[:], -1.0)
            lp = const.tile([1, C], F32)       # log_prior
            nc.scalar.activation(lp[:], pr[:], AF.Identity, bias=nb[:], scale=1.0)

            # ---- load + preprocess; fold z^2/mu^2 reductions in -------
            muT_s = const.tile([P, KT, C], F32R)    # mu^T * inv_var (rounded)
            zT_s = const.tile([P, KT, NSH], F32R)   # z^T (rounded)
            musq_acc = const.tile([1, C], F32)      # sum_d mu^2 inv_var
            nc.vector.memset(musq_acc[:], 0.0)
            zsq_acc = const.tile([P, NT], F32)      # sum_d z^2 inv_var
            nc.vector.memset(zsq_acc[:], 0.0)

            for kt in range(KT):
                rmu = rot.tile([P, C], F32, tag="rmu")
                nc.sync.dma_start(out=rmu[:], in_=muT[kt * P:(kt + 1) * P, :])
                nc.vector.tensor_scalar_mul(muT_s[:, kt, :], rmu[:],
                                            iv[:, kt:kt + 1])
                sqm = sq.tile([P, C], F32R, tag="sqm")
                nc.scalar.activation(sqm[:], rmu[:], AF.Square)
                for cj in range(CJ):
                    pmu = psS.tile([P, F], F32, tag="setup")
                    nc.tensor.matmul(pmu[0:1, :],
                                     lhsT=iv_r[:, kt:kt + 1],
                                     rhs=sqm[:, cj * F:(cj + 1) * F],
                                     start=True, stop=True)
                    nc.vector.tensor_tensor(musq_acc[:, cj * F:(cj + 1) * F],
                                            musq_acc[:, cj * F:(cj + 1) * F],
                                            pmu[0:1, :], ALU.add)

                rz = rot.tile([P, NSH], F32, tag="rz")
                nc.sync.dma_start(out=rz[:], in_=zT[kt * P:(kt + 1) * P, :])
                nc.scalar.copy(zT_s[:, kt, :], rz[:])
                pz = psS.tile([P, NT, 2], F32, tag="ptmp")
                for ni in range(NT):
                    zq = sq.tile([P, P], F32R, tag="zq")
                    nc.vector.tensor_tensor(zq[:], rz[:, ni * P:(ni + 1) * P],
                                            rz[:, ni * P:(ni + 1) * P], ALU.mult)
                    nc.tensor.matmul(pz[:, ni, :],
                                     lhsT=zq[:],
                                     rhs=iv2[:, kt, :],
                                     start=True, stop=True)
                nc.vector.tensor_tensor(zsq_acc[:], zsq_acc[:], pz[:, :, 0],
                                        ALU.add)

            # ---- biases ----------------------------------------------
            # eR[c] = log_prior[c] - 0.5*(mu_sq[c] + log_det)
            eRt = const.tile([1, C], F32)
            nc.scalar.activation(eRt[:], musq_acc[:], AF.Identity,
                                 bias=nldh[:], scale=-0.5)
            eR = const.tile([1, C], F32R)
            nc.vector.tensor_tensor(eR[:], eRt[:], lp[:], ALU.add)

            rb = const.tile([P, NT], F32)      # -0.5 * z_sq, partition layout
            nc.scalar.mul(rb[:], zsq_acc[:], -0.5)

            # broadcast eR to all partitions via rank-1 matmul
            ones_f = const.tile([1, P], F32)
            nc.vector.memset(ones_f[:], 1.0)
            ones1 = const.tile([1, P], F32R)
            nc.scalar.copy(ones1[:], ones_f[:])
            cb = const.tile([P, C], F32)
            for cj in range(CJ):
                pc = psS.tile([P, F], F32, tag="setup")
                nc.tensor.matmul(pc[:], lhsT=ones1[:],
                                 rhs=eR[:, cj * F:(cj + 1) * F],
                                 start=True, stop=True)
                nc.scalar.copy(cb[:, cj * F:(cj + 1) * F], pc[:])

            # ---- main GEMM: out = cross + rb + cb ---------------------
            for ni in range(NT):
                ot = stage.tile([P, C], F32)
                for cj in range(CJ):
                    ps = psM.tile([P, F], F32)
                    for kt in range(KT):
                        nc.tensor.matmul(
                            ps[:],
                            lhsT=zT_s[:, kt, ni * P:(ni + 1) * P],
                            rhs=muT_s[:, kt, cj * F:(cj + 1) * F],
                            start=(kt == 0), stop=(kt == KT - 1))
                    nc.scalar.activation(ot[:, cj * F:(cj + 1) * F], ps[:],
                                         AF.Identity, bias=rb[:, ni:ni + 1],
                                         scale=1.0)
                    nc.vector.tensor_tensor(ot[:, cj * F:(cj + 1) * F],
                                            ot[:, cj * F:(cj + 1) * F],
                                            cb[:, cj * F:(cj + 1) * F], ALU.add)
                nc.sync.dma_start(out=out[ni * P:(ni + 1) * P, :], in_=ot[:])

    nc.compile()
    return nc


def _get_nc():
    if "nc" not in _CACHE:
        _CACHE["nc"] = _build()
    return _CACHE["nc"]


def _in_maps(z, mu, log_cov_diag, prior_logits):
    z = np.ascontiguousarray(np.asarray(z, dtype=np.float32))
    mu = np.asarray(mu, dtype=np.float32)
    lc = np.ascontiguousarray(np.asarray(log_cov_diag, dtype=np.float32))
    pl = np.ascontiguousarray(np.asarray(prior_logits, dtype=np.float32))
    muT = np.ascontiguousarray(mu.T)
    maps = []
    for c in range(NCORES):
        zTc = np.ascontiguousarray(z[c * NSH:(c + 1) * NSH, :].T)
        maps.append({"zT": zTc, "muT": muT, "lc": lc, "prior": pl})
    return maps


def _run(z, mu, log_cov_diag, prior_logits, trace=False, **kw):
    nc = _get_nc()
    maps = _in_maps(z, mu, log_cov_diag, prior_logits)
    res = run_bass_kernel_spmd(nc, maps, list(range(NCORES)), trace=trace, **kw)
    full = np.concatenate([res.results[c]["out"] for c in range(NCORES)], axis=0)
    return full, res


def kernel(z, mu, log_cov_diag, prior_logits):
    full, _ = _run(z, mu, log_cov_diag, prior_logits)
    return full


# revision 7
# speedup vs baseline: 1.0457x; 1.0457x over previous
"""Trainium2 Bass kernel for DiagTrainableLDAHead (retrieval_knn).

out[n,c] = log_prior[c] - 0.5*(m2[n,c] + log_det)
m2[n,c]  = sum_d (z[n,d]-mu[c,d])^2 * inv_var[d]
         = z_sq[n] - 2*cross[n,c] + mu_sq[c]

=> out[n,c] = cross[n,c] + rb[n] + cb[c]
   cross = z @ (mu * inv_var).T            (GEMM, fp32r single-pass)
   rb[n] = -0.5 * sum_d z[n,d]^2 inv_var[d]
   cb[c] = log_prior[c] - 0.5*(mu_sq[c] + log_det)

Sharding: data-parallel over N across 8 NeuronCores (1024 rows each);
mu / log_cov_diag / prior_logits replicated. Forward-only: no collectives.
Host prep is layout-only (transposes so the contraction dim D sits on
SBUF partitions for both GEMM operands); all arithmetic is on-device.

Inputs stream in as column chunks (full D for a c- or n-range), chained
so early chunks complete early and the GEMM overlaps the load.
"""
import sys

sys.path.insert(0, "/opt/trn_rl_repo")

import numpy as np

import concourse.bacc as bacc
import concourse.tile as tile
from concourse import mybir
from concourse.bass_utils import run_bass_kernel_spmd

F32 = mybir.dt.float32
F32R = mybir.dt.float32r
AF = mybir.ActivationFunctionType
ALU = mybir.AluOpType

N, C, D = 8192, 2048, 512
NCORES = 8
NSH = N // NCORES          # 1024 rows per core
P = 128                    # partitions
KT = D // P                # 4 k-tiles
NT = NSH // P              # 8 n-tiles
F = 512                    # c-chunk (PSUM bank / fp32 moving max)
CJ = C // F                # 4 c-chunks

_CACHE = {}


def _build():
    nc = bacc.Bacc("TRN2", target_bir_lowering=False, debug=False,
                   enable_asserts=False, num_devices=NCORES)

    zT = nc.dram_tensor("zT", [D, NSH], F32, kind="ExternalInput").ap()
    muT = nc.dram_tensor("muT", [D, C], F32, kind="ExternalInput").ap()
    lc = nc.dram_tensor("lc", [D], F32, kind="ExternalInput").ap()
    prior = nc.dram_tensor("prior", [C], F32, kind="ExternalInput").ap()
    out = nc.dram_tensor("out", [NSH, C], F32, kind="ExternalOutput").ap()

    with tile.TileContext(nc) as tc:
        with (
            tc.tile_pool(name="const", bufs=1) as const,
            tc.tile_pool(name="rot", bufs=2) as rot,
            tc.tile_pool(name="sq", bufs=2) as sq,
            tc.tile_pool(name="stage", bufs=4) as stage,
            tc.tile_pool(name="psS", bufs=2, space="PSUM") as psS,
            tc.tile_pool(name="psM", bufs=4, space="PSUM") as psM,
        ):
            load_chain = []  # chained input DMAs: early chunks finish early

            def chained_dma(out_ap, in_ap):
                d = nc.sync.dma_start(out=out_ap, in_=in_ap)
                if load_chain:
                    tile.add_dep_helper(d.ins, load_chain[-1].ins, sync=True,
                                        reason="stage input loads")
                load_chain.append(d)
                return d

            # ---- small constants --------------------------------------
            # log_cov in partition layout [p, t] with d = t*128 + p
            lc_p = const.tile([P, KT], F32)
            chained_dma(lc_p[:], lc.rearrange("(t p) -> p t", p=P))
            lc_f = const.tile([1, D], F32)
            chained_dma(lc_f[:], lc.rearrange("(a d) -> a d", a=1))
            pr = const.tile([1, C], F32)
            chained_dma(pr[:], prior.rearrange("(a c) -> a c", a=1))

            iv = const.tile([P, KT], F32)      # exp(-lc), for scalar ops
            nc.scalar.activation(iv[:], lc_p[:], AF.Exp, scale=-1.0)
            iv_r = const.tile([P, KT], F32R)   # rounded copy, matmul operand
            nc.scalar.activation(iv_r[:], lc_p[:], AF.Exp, scale=-1.0)
            # [iv, 0] pairs: fp32r matmul needs moving free dim >= 2
            iv2 = const.tile([P, KT, 2], F32R)
            nc.scalar.activation(iv2[:, :, 0:1], lc_p[:, :, None], AF.Exp,
                                 scale=-1.0)
            nc.scalar.mul(iv2[:, :, 1:2], lc_p[:, :, None], 0.0)

            # log_det = sum(lc); computed exactly along the free dim
            ldsum = const.tile([1, 1], F32)
            nc.vector.tensor_reduce(out=ldsum[:], in_=lc_f[:],
                                    axis=mybir.AxisListType.X, op=ALU.add)
            nldh = const.tile([1, 1], F32)     # -0.5 * log_det
            nc.scalar.mul(nldh[:], ldsum[:], -0.5)

            # log_prior = prior - max - log(sum(exp(prior - max)))
            pmax = const.tile([1, 1], F32)
            nc.vector.tensor_reduce(out=pmax[:], in_=pr[:],
                                    axis=mybir.AxisListType.X, op=ALU.max)
            npmax = const.tile([1, 1], F32)
            nc.scalar.mul(npmax[:], pmax[:], -1.0)
            pexp = const.tile([1, C], F32)
            nc.scalar.activation(pexp[:], pr[:], AF.Exp, bias=npmax[:], scale=1.0)
            psum_e = const.tile([1, 1], F32)
            nc.vector.tensor_reduce(out=psum_e[:], in_=pexp[:],
                                    axis=mybir.AxisListType.X, op=ALU.add)
            lse = const.tile([1, 1], F32)
            nc.scalar.activation(lse[:], psum_e[:], AF.Ln)
            nb = const.tile([1, 1], F32)       # -(lse + pmax)
            nc.scalar.activation(nb[:], lse[:], AF.Identity, bias=pmax[:], scale=1.0)
            nc.scalar.mul(nb[:], nb[:], -1.0)
            lp = const.tile([1, C], F32)       # log_prior
            nc.scalar.activation(lp[:], pr[:], AF.Identity, bias=nb[:], scale=1.0)

            ones_f = const.tile([1, P], F32)
            nc.vector.memset(ones_f[:], 1.0)
            ones1 = const.tile([1, P], F32R)
            nc.scalar.copy(ones1[:], ones_f[:])

            # ---- streamed loads + per-chunk preprocess ----------------
            muT_s = const.tile([P, KT, C], F32R)    # mu^T * inv_var (rounded)
            zT_s = const.tile([P, KT, NSH], F32R)   # z^T (rounded)
            eRt = const.tile([1, C], F32)
            eR = const.tile([1, C], F32R)
            cb = const.tile([P, C], F32)            # eR broadcast to partitions
            rb = const.tile([P, NT], F32)           # -0.5 * z_sq

            def load_mu(cj):
                rmu = rot.tile([P, KT, F], F32, tag="rmu")
                chained_dma(rmu[:],
                            muT[:, cj * F:(cj + 1) * F]
                            .rearrange("(t p) c -> p t c", p=P))
                sqm = sq.tile([P, KT, F], F32R, tag="sqm")
                nc.scalar.activation(sqm[:], rmu[:], AF.Square)
                for kt in range(KT):
                    nc.vector.tensor_scalar_mul(
                        muT_s[:, kt, cj * F:(cj + 1) * F], rmu[:, kt, :],
                        iv[:, kt:kt + 1])
                pmu = psS.tile([P, F], F32, tag="setup")
                for kt in range(KT):
                    nc.tensor.matmul(pmu[0:1, :], lhsT=iv_r[:, kt:kt + 1],
                                     rhs=sqm[:, kt, :],
                                     start=(kt == 0), stop=(kt == KT - 1))
                # eR[c] = log_prior[c] - 0.5*(mu_sq[c] + log_det)
                nc.scalar.activation(eRt[:, cj * F:(cj + 1) * F], pmu[0:1, :],
                                     AF.Identity, bias=nldh[:], scale=-0.5)
                nc.vector.tensor_tensor(eR[:, cj * F:(cj + 1) * F],
                                        eRt[:, cj * F:(cj + 1) * F],
                                        lp[:, cj * F:(cj + 1) * F], ALU.add)
                # broadcast to all partitions via rank-1 matmul
                pc = psS.tile([P, F], F32, tag="setup")
                nc.tensor.matmul(pc[:], lhsT=ones1[:],
                                 rhs=eR[:, cj * F:(cj + 1) * F],
                                 start=True, stop=True)
                nc.scalar.copy(cb[:, cj * F:(cj + 1) * F], pc[:])

            def load_z(ni):
                rz = rot.tile([P, KT, P], F32, tag="rz")
                chained_dma(rz[:],
                            zT[:, ni * P:(ni + 1) * P]
                            .rearrange("(t p) c -> p t c", p=P))
                nc.scalar.copy(zT_s[:, :, ni * P:(ni + 1) * P], rz[:])
                zq = sq.tile([P, KT, P], F32R, tag="zq")
                nc.vector.tensor_tensor(zq[:], rz[:], rz[:], ALU.mult)
                pz = psS.tile([P, 2], F32, tag="ptmp")
                for kt in range(KT):
                    nc.tensor.matmul(pz[:], lhsT=zq[:, kt, :],
                                     rhs=iv2[:, kt, :],
                                     start=(kt == 0), stop=(kt == KT - 1))
                nc.scalar.mul(rb[:, ni:ni + 1], pz[:, 0:1], -0.5)

            # interleaved order: mu c0, z n0, mu c1, z n1, ... then z n4..n7
            for i in range(CJ):
                load_mu(i)
                load_z(i)
            for ni in range(CJ, NT):
                load_z(ni)

            # ---- main GEMM: out = cross + rb + cb ---------------------
            for ni in range(NT):
                ot = stage.tile([P, C], F32)
                for cj in range(CJ):
                    ps = psM.tile([P, F], F32)
                    for kt in range(KT):
                        nc.tensor.matmul(
                            ps[:],
                            lhsT=zT_s[:, kt, ni * P:(ni + 1) * P],
                            rhs=muT_s[:, kt, cj * F:(cj + 1) * F],
                            start=(kt == 0), stop=(kt == KT - 1))
                    nc.scalar.activation(ot[:, cj * F:(cj + 1) * F], ps[:],
                                         AF.Identity, bias=rb[:, ni:ni + 1],
                                         scale=1.0)
                    nc.vector.tensor_tensor(ot[:, cj * F:(cj + 1) * F],
                                            ot[:, cj * F:(cj + 1) * F],
                                            cb[:, cj * F:(cj + 1) * F], ALU.add)
                nc.sync.dma_start(out=out[ni * P:(ni + 1) * P, :], in_=ot[:])

    nc.compile()
    return nc


def _get_nc():
    if "nc" not in _CACHE:
        _CACHE["nc"] = _build()
    return _CACHE["nc"]


def _in_maps(z, mu, log_cov_diag, prior_logits):
    z = np.ascontiguousarray(np.asarray(z, dtype=np.float32))
    mu = np.asarray(mu, dtype=np.float32)
    lc = np.ascontiguousarray(np.asarray(log_cov_diag, dtype=np.float32))
    pl = np.ascontiguousarray(np.asarray(prior_logits, dtype=np.float32))
    muT = np.ascontiguousarray(mu.T)
    maps = []
    for c in range(NCORES):
        zTc = np.ascontiguousarray(z[c * NSH:(c + 1) * NSH, :].T)
        maps.append({"zT": zTc, "muT": muT, "lc": lc, "prior": pl})
    return maps


def _run(z, mu, log_cov_diag, prior_logits, trace=False, **kw):
    nc = _get_nc()
    maps = _in_maps(z, mu, log_cov_diag, prior_logits)
    res = run_bass_kernel_spmd(nc, maps, list(range(NCORES)), trace=trace, **kw)
    full = np.concatenate([res.results[c]["out"] for c in range(NCORES)], axis=0)
    return full, res


def kernel(z, mu, log_cov_diag, prior_logits):
    full, _ = _run(z, mu, log_cov_diag, prior_logits)
    return full


# revision 10
# speedup vs baseline: 1.2076x; 1.1549x over previous
"""Trainium2 Bass kernel for DiagTrainableLDAHead (retrieval_knn).

out[n,c] = log_prior[c] - 0.5*(m2[n,c] + log_det)
m2[n,c]  = sum_d (z[n,d]-mu[c,d])^2 * inv_var[d]
         = z_sq[n] - 2*cross[n,c] + mu_sq[c]

=> out[n,c] = cross[n,c] + rb[n] + cb[c]
   cross = z @ (mu * inv_var).T            (GEMM, fp32r single-pass)
   rb[n] = -0.5 * sum_d z[n,d]^2 inv_var[d]
   cb[c] = log_prior[c] - 0.5*(mu_sq[c] + log_det)

Sharding: data-parallel over N across 8 NeuronCores (1024 rows each);
mu / log_cov_diag / prior_logits replicated. Forward-only: no collectives.
Host prep is layout-only (transposes so the contraction dim D sits on
SBUF partitions for both GEMM operands); all arithmetic is on-device.

Inputs stream in as column chunks (full D for a c- or n-range), chained
so early chunks complete early and the GEMM overlaps the load.
"""
import sys

sys.path.insert(0, "/opt/trn_rl_repo")

import numpy as np

import concourse.bacc as bacc
import concourse.tile as tile
from concourse import mybir
from concourse.bass_utils import run_bass_kernel_spmd

F32 = mybir.dt.float32
F32R = mybir.dt.float32r
AF = mybir.ActivationFunctionType
ALU = mybir.AluOpType

N, C, D = 8192, 2048, 512
NCORES = 8
NSH = N // NCORES          # 1024 rows per core
P = 128                    # partitions
KT = D // P                # 4 k-tiles
NT = NSH // P              # 8 n-tiles
F = 512                    # c-chunk (PSUM bank / fp32 moving max)
CJ = C // F                # 4 c-chunks

_CACHE = {}


def _build():
    nc = bacc.Bacc("TRN2", target_bir_lowering=False, debug=False,
                   enable_asserts=False, num_devices=NCORES)

    zT = nc.dram_tensor("zT", [D, NSH], F32, kind="ExternalInput").ap()
    muT = nc.dram_tensor("muT", [D, C], F32, kind="ExternalInput").ap()
    lc = nc.dram_tensor("lc", [D], F32, kind="ExternalInput").ap()
    prior = nc.dram_tensor("prior", [C], F32, kind="ExternalInput").ap()
    out = nc.dram_tensor("out", [NSH, C], F32, kind="ExternalOutput").ap()

    with tile.TileContext(nc) as tc:
        with (
            tc.tile_pool(name="const", bufs=1) as const,
            tc.tile_pool(name="rot", bufs=2) as rot,
            tc.tile_pool(name="sq", bufs=2) as sq,
            tc.tile_pool(name="stage", bufs=4) as stage,
            tc.tile_pool(name="psS", bufs=2, space="PSUM") as psS,
            tc.tile_pool(name="psM", bufs=4, space="PSUM") as psM,
        ):
            # ---- small constants --------------------------------------
            lc_f = const.tile([1, D], F32)
            nc.sync.dma_start(out=lc_f[:], in_=lc.rearrange("(a d) -> a d", a=1))
            pr = const.tile([1, C], F32)
            nc.sync.dma_start(out=pr[:], in_=prior.rearrange("(a c) -> a c", a=1))

            # log_cov in partition layout [p, t] (d = t*128 + p) via PE
            # transposes — a strided DMA gather here costs ~3us of
            # descriptor generation on the sequencer.
            id1 = const.tile([1, 1], F32)
            nc.vector.memset(id1[:], 1.0)
            plc = psS.tile([P, KT], F32, tag="ptmp")
            for kt in range(KT):
                nc.tensor.transpose(plc[:, kt:kt + 1],
                                    lc_f[:, kt * P:(kt + 1) * P], id1[:])
            lc_p = const.tile([P, KT], F32)
            nc.scalar.copy(lc_p[:], plc[:])

            iv = const.tile([P, KT], F32)      # exp(-lc), for scalar ops
            nc.scalar.activation(iv[:], lc_p[:], AF.Exp, scale=-1.0)
            iv_r = const.tile([P, KT], F32R)   # rounded copy, matmul operand
            nc.scalar.activation(iv_r[:], lc_p[:], AF.Exp, scale=-1.0)
            # [iv, 0] pairs: fp32r matmul needs moving free dim >= 2
            iv2 = const.tile([P, KT, 2], F32R)
            nc.scalar.activation(iv2[:, :, 0:1], lc_p[:, :, None], AF.Exp,
                                 scale=-1.0)
            nc.scalar.mul(iv2[:, :, 1:2], lc_p[:, :, None], 0.0)

            # log_det = sum(lc); computed exactly along the free dim
            ldsum = const.tile([1, 1], F32)
            nc.vector.tensor_reduce(out=ldsum[:], in_=lc_f[:],
                                    axis=mybir.AxisListType.X, op=ALU.add)
            nldh = const.tile([1, 1], F32)     # -0.5 * log_det
            nc.scalar.mul(nldh[:], ldsum[:], -0.5)

            # log_prior = prior - max - log(sum(exp(prior - max)))
            pmax = const.tile([1, 1], F32)
            nc.vector.tensor_reduce(out=pmax[:], in_=pr[:],
                                    axis=mybir.AxisListType.X, op=ALU.max)
            npmax = const.tile([1, 1], F32)
            nc.scalar.mul(npmax[:], pmax[:], -1.0)
            pexp = const.tile([1, C], F32)
            nc.scalar.activation(pexp[:], pr[:], AF.Exp, bias=npmax[:], scale=1.0)
            psum_e = const.tile([1, 1], F32)
            nc.vector.tensor_reduce(out=psum_e[:], in_=pexp[:],
                                    axis=mybir.AxisListType.X, op=ALU.add)
            lse = const.tile([1, 1], F32)
            nc.scalar.activation(lse[:], psum_e[:], AF.Ln)
            nb = const.tile([1, 1], F32)       # -(lse + pmax)
            nc.scalar.activation(nb[:], lse[:], AF.Identity, bias=pmax[:], scale=1.0)
            nc.scalar.mul(nb[:], nb[:], -1.0)
            lp = const.tile([1, C], F32)       # log_prior
            nc.scalar.activation(lp[:], pr[:], AF.Identity, bias=nb[:], scale=1.0)

            ones_f = const.tile([1, P], F32)
            nc.vector.memset(ones_f[:], 1.0)
            ones1 = const.tile([1, P], F32R)
            nc.scalar.copy(ones1[:], ones_f[:])

            # ---- streamed loads + per-chunk preprocess ----------------
            muT_s = const.tile([P, KT, C], F32R)    # mu^T * inv_var (rounded)
            zT_s = const.tile([P, KT, NSH], F32R)   # z^T (rounded)
            eRt = const.tile([1, C], F32)
            eR = const.tile([1, C], F32R)
            cb = const.tile([P, C], F32)            # eR broadcast to partitions
            rb = const.tile([P, NT], F32)           # -0.5 * z_sq

            def load_mu(cj):
                rmu = rot.tile([P, KT, F], F32, tag="rmu")
                nc.sync.dma_start(out=rmu[:],
                                  in_=muT[:, cj * F:(cj + 1) * F]
                                  .rearrange("(t p) c -> p t c", p=P))
                sqm = sq.tile([P, KT, F], F32R, tag="sqm")
                nc.scalar.activation(sqm[:], rmu[:], AF.Square)
                for kt in range(KT):
                    nc.vector.tensor_scalar_mul(
                        muT_s[:, kt, cj * F:(cj + 1) * F], rmu[:, kt, :],
                        iv[:, kt:kt + 1])
                pmu = psS.tile([P, F], F32, tag="setup")
                for kt in range(KT):
                    nc.tensor.matmul(pmu[0:1, :], lhsT=iv_r[:, kt:kt + 1],
                                     rhs=sqm[:, kt, :],
                                     start=(kt == 0), stop=(kt == KT - 1))
                # eR[c] = log_prior[c] - 0.5*(mu_sq[c] + log_det)
                nc.scalar.activation(eRt[:, cj * F:(cj + 1) * F], pmu[0:1, :],
                                     AF.Identity, bias=nldh[:], scale=-0.5)
                nc.vector.tensor_tensor(eR[:, cj * F:(cj + 1) * F],
                                        eRt[:, cj * F:(cj + 1) * F],
                                        lp[:, cj * F:(cj + 1) * F], ALU.add)
                # broadcast to all partitions via rank-1 matmul
                pc = psS.tile([P, F], F32, tag="setup")
                nc.tensor.matmul(pc[:], lhsT=ones1[:],
                                 rhs=eR[:, cj * F:(cj + 1) * F],
                                 start=True, stop=True)
                nc.scalar.copy(cb[:, cj * F:(cj + 1) * F], pc[:])

            def load_z(ni):
                rz = rot.tile([P, KT, P], F32, tag="rz")
                nc.sync.dma_start(out=rz[:],
                                  in_=zT[:, ni * P:(ni + 1) * P]
                                  .rearrange("(t p) c -> p t c", p=P))
                nc.scalar.copy(zT_s[:, :, ni * P:(ni + 1) * P], rz[:])
                zq = sq.tile([P, KT, P], F32R, tag="zq")
                nc.vector.tensor_tensor(zq[:], rz[:], rz[:], ALU.mult)
                pz = psS.tile([P, 2], F32, tag="ptmp")
                for kt in range(KT):
                    nc.tensor.matmul(pz[:], lhsT=zq[:, kt, :],
                                     rhs=iv2[:, kt, :],
                                     start=(kt == 0), stop=(kt == KT - 1))
                nc.scalar.mul(rb[:, ni:ni + 1], pz[:, 0:1], -0.5)

            # interleaved order: mu c0, z n0, mu c1, z n1, ... then z n4..n7
            for i in range(CJ):
                load_mu(i)
                load_z(i)
            for ni in range(CJ, NT):
                load_z(ni)

            # ---- main GEMM: out = cross + rb + cb ---------------------
            for ni in range(NT):
                ot = stage.tile([P, C], F32)
                for cj in range(CJ):
                    ps = psM.tile([P, F], F32)
                    for kt in range(KT):
                        nc.tensor.matmul(
                            ps[:],
                            lhsT=zT_s[:, kt, ni * P:(ni + 1) * P],
                            rhs=muT_s[:, kt, cj * F:(cj + 1) * F],
                            start=(kt == 0), stop=(kt == KT - 1))
                    nc.scalar.activation(ot[:, cj * F:(cj + 1) * F], ps[:],
                                         AF.Identity, bias=rb[:, ni:ni + 1],
                                         scale=1.0)
                    nc.vector.tensor_tensor(ot[:, cj * F:(cj + 1) * F],
                                            ot[:, cj * F:(cj + 1) * F],
                                            cb[:, cj * F:(cj + 1) * F], ALU.add)
                nc.sync.dma_start(out=out[ni * P:(ni + 1) * P, :], in_=ot[:])

    nc.compile()
    return nc


def _get_nc():
    if "nc" not in _CACHE:
        _CACHE["nc"] = _build()
    return _CACHE["nc"]


def _in_maps(z, mu, log_cov_diag, prior_logits):
    z = np.ascontiguousarray(np.asarray(z, dtype=np.float32))
    mu = np.asarray(mu, dtype=np.float32)
    lc = np.ascontiguousarray(np.asarray(log_cov_diag, dtype=np.float32))
    pl = np.ascontiguousarray(np.asarray(prior_logits, dtype=np.float32))
    muT = np.ascontiguousarray(mu.T)
    maps = []
    for c in range(NCORES):
        zTc = np.ascontiguousarray(z[c * NSH:(c + 1) * NSH, :].T)
        maps.append({"zT": zTc, "muT": muT, "lc": lc, "prior": pl})
    return maps


def _run(z, mu, log_cov_diag, prior_logits, trace=False, **kw):
    nc = _get_nc()
    maps = _in_maps(z, mu, log_cov_diag, prior_logits)
    res = run_bass_kernel_spmd(nc, maps, list(range(NCORES)), trace=trace, **kw)
    full = np.concatenate([res.results[c]["out"] for c in range(NCORES)], axis=0)
    return full, res


def kernel(z, mu, log_cov_diag, prior_logits):
    full, _ = _run(z, mu, log_cov_diag, prior_logits)
    return full


# revision 13
# speedup vs baseline: 1.2255x; 1.0148x over previous
"""Trainium2 Bass kernel for DiagTrainableLDAHead (retrieval_knn).

out[n,c] = log_prior[c] - 0.5*(m2[n,c] + log_det)
m2[n,c]  = sum_d (z[n,d]-mu[c,d])^2 * inv_var[d]
         = z_sq[n] - 2*cross[n,c] + mu_sq[c]

=> out[n,c] = cross[n,c] + rb[n] + cb[c]
   cross = z @ (mu * inv_var).T            (GEMM, fp32r single-pass)
   rb[n] = -0.5 * sum_d z[n,d]^2 inv_var[d]
   cb[c] = log_prior[c] - 0.5*(mu_sq[c] + log_det)

Sharding: data-parallel over N across 8 NeuronCores (1024 rows each);
mu / log_cov_diag / prior_logits replicated. Forward-only: no collectives.
Host prep is layout-only (transposes so the contraction dim D sits on
SBUF partitions for both GEMM operands); all arithmetic is on-device.

Inputs stream in as column chunks (full D for a c- or n-range), chained
so early chunks complete early and the GEMM overlaps the load.
"""
import sys

sys.path.insert(0, "/opt/trn_rl_repo")

import numpy as np

import concourse.bacc as bacc
import concourse.tile as tile
from concourse import mybir
from concourse.bass_utils import run_bass_kernel_spmd

F32 = mybir.dt.float32
F32R = mybir.dt.float32r
AF = mybir.ActivationFunctionType
ALU = mybir.AluOpType

N, C, D = 8192, 2048, 512
NCORES = 8
NSH = N // NCORES          # 1024 rows per core
P = 128                    # partitions
KT = D // P                # 4 k-tiles
NT = NSH // P              # 8 n-tiles
F = 512                    # c-chunk (PSUM bank / fp32 moving max)
CJ = C // F                # 4 c-chunks

_CACHE = {}


def _build():
    nc = bacc.Bacc("TRN2", target_bir_lowering=False, debug=False,
                   enable_asserts=False, num_devices=NCORES)

    zT = nc.dram_tensor("zT", [D, NSH], F32, kind="ExternalInput").ap()
    muT = nc.dram_tensor("muT", [D, C], F32, kind="ExternalInput").ap()
    lc = nc.dram_tensor("lc", [D], F32, kind="ExternalInput").ap()
    prior = nc.dram_tensor("prior", [C], F32, kind="ExternalInput").ap()
    out = nc.dram_tensor("out", [NSH, C], F32, kind="ExternalOutput").ap()

    with tile.TileContext(nc) as tc:
        with (
            tc.tile_pool(name="const", bufs=1) as const,
            tc.tile_pool(name="rot", bufs=2) as rot,
            tc.tile_pool(name="sq", bufs=2) as sq,
            tc.tile_pool(name="stage", bufs=4) as stage,
            tc.tile_pool(name="psS", bufs=2, space="PSUM") as psS,
            tc.tile_pool(name="psM", bufs=4, space="PSUM") as psM,
        ):
            # ---- small constants --------------------------------------
            # (issued on the scalar queue so the sync queue's first issue
            # is the first big mu chunk)
            lc_f = const.tile([1, D], F32)
            nc.scalar.dma_start(out=lc_f[:], in_=lc.rearrange("(a d) -> a d", a=1))
            pr = const.tile([1, C], F32)
            nc.scalar.dma_start(out=pr[:], in_=prior.rearrange("(a c) -> a c", a=1))

            # log_cov in partition layout [p, t] (d = t*128 + p) via PE
            # transposes — a strided DMA gather here costs ~3us of
            # descriptor generation on the sequencer.
            id1 = const.tile([1, 1], F32)
            nc.vector.memset(id1[:], 1.0)
            plc = psS.tile([P, KT], F32, tag="ptmp")
            for kt in range(KT):
                nc.tensor.transpose(plc[:, kt:kt + 1],
                                    lc_f[:, kt * P:(kt + 1) * P], id1[:])
            lc_p = const.tile([P, KT], F32)
            nc.scalar.copy(lc_p[:], plc[:])

            iv = const.tile([P, KT], F32)      # exp(-lc), for scalar ops
            nc.scalar.activation(iv[:], lc_p[:], AF.Exp, scale=-1.0)
            iv_r = const.tile([P, KT], F32R)   # rounded copy, matmul operand
            nc.scalar.activation(iv_r[:], lc_p[:], AF.Exp, scale=-1.0)
            # [iv, 0] pairs: fp32r matmul needs moving free dim >= 2
            iv2 = const.tile([P, KT, 2], F32R)
            nc.scalar.activation(iv2[:, :, 0:1], lc_p[:, :, None], AF.Exp,
                                 scale=-1.0)
            nc.scalar.mul(iv2[:, :, 1:2], lc_p[:, :, None], 0.0)

            # log_det = sum(lc); computed exactly along the free dim
            ldsum = const.tile([1, 1], F32)
            nc.vector.tensor_reduce(out=ldsum[:], in_=lc_f[:],
                                    axis=mybir.AxisListType.X, op=ALU.add)
            nldh = const.tile([1, 1], F32)     # -0.5 * log_det
            nc.scalar.mul(nldh[:], ldsum[:], -0.5)

            # log_prior = prior - max - log(sum(exp(prior - max)))
            pmax = const.tile([1, 1], F32)
            nc.vector.tensor_reduce(out=pmax[:], in_=pr[:],
                                    axis=mybir.AxisListType.X, op=ALU.max)
            npmax = const.tile([1, 1], F32)
            nc.scalar.mul(npmax[:], pmax[:], -1.0)
            pexp = const.tile([1, C], F32)
            nc.scalar.activation(pexp[:], pr[:], AF.Exp, bias=npmax[:], scale=1.0)
            psum_e = const.tile([1, 1], F32)
            nc.vector.tensor_reduce(out=psum_e[:], in_=pexp[:],
                                    axis=mybir.AxisListType.X, op=ALU.add)
            lse = const.tile([1, 1], F32)
            nc.scalar.activation(lse[:], psum_e[:], AF.Ln)
            nb = const.tile([1, 1], F32)       # -(lse + pmax)
            nc.scalar.activation(nb[:], lse[:], AF.Identity, bias=pmax[:], scale=1.0)
            nc.scalar.mul(nb[:], nb[:], -1.0)
            lp = const.tile([1, C], F32)       # log_prior
            nc.scalar.activation(lp[:], pr[:], AF.Identity, bias=nb[:], scale=1.0)

            ones_f = const.tile([1, P], F32)
            nc.vector.memset(ones_f[:], 1.0)
            ones1 = const.tile([1, P], F32R)
            nc.scalar.copy(ones1[:], ones_f[:])

            # ---- streamed loads + per-chunk preprocess ----------------
            muT_s = const.tile([P, KT, C], F32R)    # mu^T * inv_var (rounded)
            zT_s = const.tile([P, KT, NSH], F32R)   # z^T (rounded)
            eRt = const.tile([1, C], F32)
            eR = const.tile([1, C], F32R)
            cb = const.tile([P, C], F32)            # eR broadcast to partitions
            rb = const.tile([P, NT], F32)           # -0.5 * z_sq

            def load_mu(cj):
                rmu = rot.tile([P, KT, F], F32, tag="rmu")
                nc.sync.dma_start(out=rmu[:],
                                  in_=muT[:, cj * F:(cj + 1) * F]
                                  .rearrange("(t p) c -> p t c", p=P))
                for kt in range(KT):
                    nc.vector.tensor_scalar_mul(
                        muT_s[:, kt, cj * F:(cj + 1) * F], rmu[:, kt, :],
                        iv[:, kt:kt + 1])
                sqm = sq.tile([P, KT, F], F32R, tag="sqm")
                nc.scalar.activation(sqm[:], rmu[:], AF.Square)
                pmu = psS.tile([P, F], F32, tag="setup")
                for kt in range(KT):
                    nc.tensor.matmul(pmu[0:1, :], lhsT=iv_r[:, kt:kt + 1],
                                     rhs=sqm[:, kt, :],
                                     start=(kt == 0), stop=(kt == KT - 1))
                # eR[c] = log_prior[c] - 0.5*(mu_sq[c] + log_det)
                nc.scalar.activation(eRt[:, cj * F:(cj + 1) * F], pmu[0:1, :],
                                     AF.Identity, bias=nldh[:], scale=-0.5)
                nc.vector.tensor_tensor(eR[:, cj * F:(cj + 1) * F],
                                        eRt[:, cj * F:(cj + 1) * F],
                                        lp[:, cj * F:(cj + 1) * F], ALU.add)
                # broadcast to all partitions via rank-1 matmul
                pc = psS.tile([P, F], F32, tag="setup")
                nc.tensor.matmul(pc[:], lhsT=ones1[:],
                                 rhs=eR[:, cj * F:(cj + 1) * F],
                                 start=True, stop=True)
                nc.scalar.copy(cb[:, cj * F:(cj + 1) * F], pc[:])

            def load_z(ni):
                rz = rot.tile([P, KT, P], F32, tag="rz")
                nc.sync.dma_start(out=rz[:],
                                  in_=zT[:, ni * P:(ni + 1) * P]
                                  .rearrange("(t p) c -> p t c", p=P))
                nc.scalar.copy(zT_s[:, :, ni * P:(ni + 1) * P], rz[:])
                zq = sq.tile([P, KT, P], F32R, tag="zq")
                nc.vector.tensor_tensor(zq[:], rz[:], rz[:], ALU.mult)
                pz = psS.tile([P, 2], F32, tag="ptmp")
                for kt in range(KT):
                    nc.tensor.matmul(pz[:], lhsT=zq[:, kt, :],
                                     rhs=iv2[:, kt, :],
                                     start=(kt == 0), stop=(kt == KT - 1))
                nc.scalar.mul(rb[:, ni:ni + 1], pz[:, 0:1], -0.5)

            # interleaved order: mu c0, z n0, mu c1, z n1, ... then z n4..n7
            for i in range(CJ):
                load_mu(i)
                load_z(i)
            for ni in range(CJ, NT):
                load_z(ni)

            # ---- main GEMM: out = cross + rb + cb ---------------------
            for ni in range(NT):
                ot = stage.tile([P, C], F32)
                for cj in range(CJ):
                    ps = psM.tile([P, F], F32)
                    for kt in range(KT):
                        nc.tensor.matmul(
                            ps[:],
                            lhsT=zT_s[:, kt, ni * P:(ni + 1) * P],
                            rhs=muT_s[:, kt, cj * F:(cj + 1) * F],
                            start=(kt == 0), stop=(kt == KT - 1))
                    nc.scalar.activation(ot[:, cj * F:(cj + 1) * F], ps[:],
                                         AF.Identity, bias=rb[:, ni:ni + 1],
                                         scale=1.0)
                    nc.vector.tensor_tensor(ot[:, cj * F:(cj + 1) * F],
                                            ot[:, cj * F:(cj + 1) * F],
                                            cb[:, cj * F:(cj + 1) * F], ALU.add)
                    if cj == 1:
                        nc.sync.dma_start(out=out[ni * P:(ni + 1) * P, 0:2 * F],
                                          in_=ot[:, 0:2 * F])
                nc.sync.dma_start(out=out[ni * P:(ni + 1) * P, 2 * F:C],
                                  in_=ot[:, 2 * F:C])

    nc.compile()
    return nc


def _get_nc():
    if "nc" not in _CACHE:
        _CACHE["nc"] = _build()
    return _CACHE["nc"]


def _in_maps(z, mu, log_cov_diag, prior_logits):
    z = np.ascontiguousarray(np.asarray(z, dtype=np.float32))
    mu = np.asarray(mu, dtype=np.float32)
    lc = np.ascontiguousarray(np.asarray(log_cov_diag, dtype=np.float32))
    pl = np.ascontiguousarray(np.asarray(prior_logits, dtype=np.float32))
    muT = np.ascontiguousarray(mu.T)
    maps = []
    for c in range(NCORES):
        zTc = np.ascontiguousarray(z[c * NSH:(c + 1) * NSH, :].T)
        maps.append({"zT": zTc, "muT": muT, "lc": lc, "prior": pl})
    return maps


def _run(z, mu, log_cov_diag, prior_logits, trace=False, **kw):
    nc = _get_nc()
    maps = _in_maps(z, mu, log_cov_diag, prior_logits)
    res = run_bass_kernel_spmd(nc, maps, list(range(NCORES)), trace=trace, **kw)
    full = np.concatenate([res.results[c]["out"] for c in range(NCORES)], axis=0)
    return full, res


def kernel(z, mu, log_cov_diag, prior_logits):
    full, _ = _run(z, mu, log_cov_diag, prior_logits)
    return full
